# revision 28
# baseline (speedup 1.0000x reference)
"""Trainium2 Bass kernel for the SLAYER-style 2-layer spiking MLP.

Reference computation (per batch element n):
    flat   = input.reshape(64, 3072)
    a1     = flat @ w1.T                      (constant over time)
    u1[t]  = a1 * c[t]          where c = cumsum(srm kernel)  (PSP of a
             time-constant input is just a ramp scale)
    s1     = spike_scan(u1)     sequential threshold w/ refractory feedback
    a2[t]  = w2 @ s1[:, t]
    u2     = psp(a2)            (true temporal conv, srm kernel)
    out    = spike_scan(u2)

Refractory feedback is an exact order-2 IIR (kernel rk[d] = -20 d e^{1-d}):
    P[t] = q*P[t-1] + s[t-1];  R[t] = q*R[t-1] + P[t]   (q = e^-1)
    spike:  (R + 0.5) <= u/20
Each fused scan step is 3 scalar_tensor_tensor DVE ops over a [104, 33]
tile holding both layers (layer 2 rides along lagged LAG steps).

Schedule highlights vs the naive version:
  * w1 / flat are host-packed so each DMA moves long contiguous
    per-partition lines (128 descriptors instead of 3072).
  * The u1/20 threshold rows are produced per-step on the otherwise-idle
    Scalar engine (activation Copy, scale=c20[t]) instead of one huge
    broadcast-AP DVE op that serialized the whole prologue.
  * PSUM evacuations ride the scalar stream at delayed slots so they
    never block threshold-row production.
  * fc2 re-stack uses a shared eye(10) stationary with per-n PSUM
    partition-offset writes (1 LDWEIGHTS instead of 8 full selector
    loads per block).

Sharding: data-parallel over batch, 8 elements per core, weights replicated.
"""

import numpy as np

NB = 8            # batch elements per core
T = 100           # timesteps
B = 16            # pipeline block size
LAG = 32          # layer-2 ride-along lag (>= block + pipeline latency)
TF = T + LAG      # fused scan steps
NCOL = 33         # 32 layer-1 columns (4 chunks x 8 batch) + 1 layer-2 column
PMAX = 104        # padded partition count per o-chunk
MC = [103, 103, 102, 102]      # o-chunk sizes (sum = 410)
OFF = [0, 103, 206, 308]
KT = 24           # 3072 / 128 k-tiles
KCS = [2, 6, 8, 8]  # k-tiles per w1 DMA chunk (small first chunk)
NO1 = 410
NO2 = 10
DELAY = 16        # DVE-stream slots after block end before W1/W2 scans
EVAC1_DELAY = 8   # stream slots after block end before PSUM evac 1
EVAC2_DELAY = 15  # (selector MMs are emitted right after evac1)
UCHUNK = 12       # threshold-surface timesteps per gpsimd instruction
OSPLIT = 88       # output columns staged/DMA'd early, overlapping the tail

_CACHE = {}


def _consts():
    q = float(np.float32(np.exp(-1.0)))          # refractory ratio
    p = float(np.float32(np.exp(-0.1)))          # SRM ratio
    k2 = float(np.float32(np.exp(1.0) / 200.0))  # a2 pre-scale: u2/20 = sum
    t = np.arange(T, dtype=np.float64)
    srm = (t / 10.0) * np.exp(1.0 - t / 10.0)
    c20 = (np.cumsum(srm) / 20.0).astype(np.float32)
    return q, p, k2, c20


def build():
    import concourse.bass as bass
    import concourse.bacc as bacc
    import concourse.mybir as mybir
    from concourse import tile

    f32 = mybir.dt.float32
    Alu = mybir.AluOpType
    Act = mybir.ActivationFunctionType
    q, p, k2, c20 = _consts()

    nc = bacc.Bacc("TRN2", target_bir_lowering=False, debug=False, num_devices=8)

    w1p_d = nc.dram_tensor("w1p", [128, KT * NO1], f32, kind="ExternalInput")
    fTp_d = nc.dram_tensor("fTp", [128, KT * NB], f32, kind="ExternalInput")
    w2p_d = nc.dram_tensor("w2p", [PMAX, 4, NO2], f32, kind="ExternalInput")
    pc_d = nc.dram_tensor("pconst", [84, B], f32, kind="ExternalInput")
    eye_d = nc.dram_tensor("eye8", [NB, NB], f32, kind="ExternalInput")
    sel_d = nc.dram_tensor("sel32", [NO2, 3, 32], f32, kind="ExternalInput")
    c20_d = nc.dram_tensor("c20rep", [PMAX, T], f32, kind="ExternalInput")
    out_d = nc.dram_tensor("out", [84, T], f32, kind="ExternalOutput")

    with tile.TileContext(nc) as tc:
        with (
            tc.tile_pool(name="pers", bufs=1) as pool,
            tc.tile_pool(name="ps1", bufs=1, space="PSUM") as ps1,
            tc.tile_pool(name="ps2", bufs=2, space="PSUM") as ps2,
        ):
            w1sb = pool.tile([128, KT, NO1], f32, tag="w1sb")
            fTsb = pool.tile([128, KT, NB], f32, tag="fTsb")
            w2sb = pool.tile([PMAX, 4, NO2], f32, tag="w2sb")
            pcsb = pool.tile([84, B], f32, tag="pcsb")
            eyesb = pool.tile([NB, NB], f32, tag="eyesb")
            selsb = pool.tile([NO2, 3, 32], f32, tag="selsb")
            a1rsb = pool.tile([NB, NO1], f32, tag="a1rsb")
            A1 = pool.tile([PMAX, 32], f32, tag="A1")
            Up = pool.tile([PMAX, NCOL, TF], f32, tag="Up")
            c20sb = pool.tile([PMAX, T], f32, tag="c20sb")
            # mega-tile: spike history S (TF+1 slots of NCOL) followed by the
            # IIR state [P(NCOL) | R(NCOL)].
            SW = (TF + 1) * NCOL
            M = pool.tile([PMAX, SW + 2 * NCOL], f32, tag="M")
            a2tmp = pool.tile([NO2, B, NB], f32, tag="a2tmp")
            a2s = pool.tile([84, T + 1], f32, tag="a2s")
            W1 = pool.tile([84, T + 1], f32, tag="W1")
            W2 = pool.tile([84, T + 1], f32, tag="W2")
            ostage = pool.tile([84, T], f32, tag="ostage")
            gsc = pool.tile([1, 4], f32, tag="gsc")

            # ---- input DMAs: flat first (gates fc1's stationary), then w1
            # chunks (small chunk 0 so fc1 starts early); constants on sync ----
            nc.gpsimd.dma_start(fTsb[:], fTp_d[:])
            ko = 0
            for kc in KCS:
                nc.gpsimd.dma_start(
                    w1sb[:, ko:ko + kc, :],
                    w1p_d[:, ko * NO1:(ko + kc) * NO1],
                )
                ko += kc
            nc.sync.dma_start(w2sb[:], w2p_d[:])
            nc.sync.dma_start(pcsb[:], pc_d[:])
            nc.sync.dma_start(eyesb[:], eye_d[:])
            nc.sync.dma_start(selsb[:], sel_d[:])
            nc.sync.dma_start(c20sb[:], c20_d[:])

            # ---- state init (rides during DMA) ----
            nc.vector.memset(M[:, 0:NCOL], 0.0)            # S slot 0
            nc.vector.memset(M[:, SW:SW + 2 * NCOL], 0.0)  # P | R
            nc.vector.memset(A1[:], 0.0)
            # layer-2 u column (incl. garbage rows) — NaN insurance
            UW = Up.ap[0][0]
            nc.gpsimd.memset(Up[:, 32, :], 0.0)
            nc.gpsimd.memset(
                bass.AP(Up.tensor, Up.offset, [[UW, PMAX], [TF, NCOL]]), 0.0
            )
            nc.gpsimd.memset(a2s[:, 0:1], 0.0)
            nc.gpsimd.memset(W1[:, 0:1], 0.0)
            nc.gpsimd.memset(W2[:, 0:1], 0.0)

            # ---- fc1: a1row[n, o] = flat @ w1.T, accumulated over k ----
            a1row = ps1.tile([NB, NO1], f32, tag="a1row", name="a1row")
            for k in range(KT):
                nc.tensor.matmul(
                    a1row[:], fTsb[:, k, :], w1sb[:, k, :],
                    start=(k == 0), stop=(k == KT - 1),
                )
            nc.vector.tensor_copy(a1rsb[:], a1row[:])
            # transpose to scan layout: A1[o_chunk, (c, n)]
            for c in range(4):
                a1tp = ps1.tile([PMAX, NB], f32, tag=f"a1tp{c % 2}", name="a1tp")
                nc.tensor.transpose(
                    a1tp[0:MC[c], :],
                    a1rsb[0:NB, OFF[c]:OFF[c] + MC[c]],
                    eyesb[:],
                )
                nc.vector.tensor_copy(
                    A1[0:MC[c], c * NB:(c + 1) * NB], a1tp[0:MC[c], :]
                )

            # ---- block/scalar-stream schedules ----
            blocks = []
            for b in range((T + B - 1) // B):
                blocks.append((b * B, min((b + 1) * B, T)))
            blk_at = {tb1 - 1: (bi, tb0, tb1)
                      for bi, (tb0, tb1) in enumerate(blocks)}
            wscan_at = {}
            for bi, (tb0, tb1) in enumerate(blocks):
                wscan_at.setdefault(min(tb1 - 1 + DELAY, T - 1), []).append(bi)
            evac_at = {}
            for bi, (tb0, tb1) in enumerate(blocks):
                evac_at.setdefault(min(tb1 - 1 + EVAC1_DELAY, T - 1), []).append(
                    (bi, 1))
                evac_at.setdefault(min(tb1 - 1 + EVAC2_DELAY, T - 1), []).append(
                    (bi, 2))

            a2ps_t = {}
            a2r_t = {}

            def emit_evac(bi, which):
                tb0, tb1 = blocks[bi]
                blk = tb1 - tb0
                if which == 1:
                    nc.scalar.activation(
                        a2tmp[:, 0:blk, :], a2ps_t[bi][:, 0:blk, :],
                        Act.Copy, scale=k2,
                    )
                else:
                    nc.scalar.activation(
                        a2s[:, tb0 + 1:tb1 + 1], a2r_t[bi][0:84, 0:blk],
                        Act.Copy,
                    )

            def emit_restack(bi):
                tb0, tb1 = blocks[bi]
                blk = tb1 - tb0
                # re-stack to gapped rows (r = 32g+10j+o2): shared shifted
                # identities, PSUM partition bases 0/32/64
                a2r = ps2.tile([96, B], f32, tag="a2r", name="a2r")
                a2r_t[bi] = a2r
                for g in range(3):
                    nj = min(3, NB - 3 * g)
                    for j in range(nj):
                        nc.tensor.matmul(
                            a2r[32 * g:32 * g + 32, 0:blk],
                            selsb[:, j, :],
                            a2tmp[:, 0:blk, 3 * g + j],
                            start=(j == 0), stop=(j == nj - 1),
                        )

            def emit_wscan(bi):
                tb0, tb1 = blocks[bi]
                blk = tb1 - tb0
                # W1[t] = (a2s[t-1] + W1[t-1]) * p   (hardware scan)
                nc.vector.tensor_tensor_scan(
                    W1[:, tb0 + 1:tb1 + 1], a2s[:, tb0:tb1], pcsb[:, 0:blk],
                    W1[:, tb0:tb0 + 1], Alu.add, Alu.mult,
                )
                nc.vector.tensor_tensor_scan(
                    W2[:, tb0 + 1:tb1 + 1], W1[:, tb0:tb1], pcsb[:, 0:blk],
                    W2[:, tb0:tb0 + 1], Alu.add, Alu.mult,
                )
                # u2/20 = W1 + W2 -> layer-2 column of Up, lagged by LAG
                nc.gpsimd.tensor_tensor(
                    Up[0:84, 32, tb0 + LAG:tb1 + LAG],
                    W1[:, tb0 + 1:tb1 + 1], W2[:, tb0 + 1:tb1 + 1], Alu.add,
                )

            # ---- fused scan: layer-1 at step tau, layer-2 at tau-LAG ----
            MW = M.ap[0][0]          # mega-tile row stride (elements)
            MOFF = M.offset

            def m_ap(off, dims, parts=PMAX):
                return bass.AP(M.tensor, MOFF + off, [[MW, parts]] + dims)

            def emit_uchunk(g0):
                if g0 >= T:
                    return
                g1 = min(g0 + UCHUNK, T)
                w = g1 - g0
                # Up[p, j, t] = A1[p, j] * c20[t] — broadcast outer product
                nc.gpsimd.tensor_tensor(
                    Up[:, 0:32, g0:g1],
                    A1[:].unsqueeze(2).broadcast_to([PMAX, 32, w]),
                    c20sb[:, g0:g1].unsqueeze(1).broadcast_to([PMAX, 32, w]),
                    Alu.mult,
                )

            # t=0 column is memset; t=1..5 are produced by the DVE inside
            # the scan loop (no cross-engine wait); gpsimd covers t >= 6
            def emit_urange(g0, g1):
                w = g1 - g0
                nc.gpsimd.tensor_tensor(
                    Up[:, 0:32, g0:g1],
                    A1[:].unsqueeze(2).broadcast_to([PMAX, 32, w]),
                    c20sb[:, g0:g1].unsqueeze(1).broadcast_to([PMAX, 32, w]),
                    Alu.mult,
                )

            emit_urange(6, 18)

            for tau in range(TF):
                if tau < T:
                    pp = m_ap(SW, [[1, NCOL]])
                    rr = m_ap(SW + NCOL, [[1, NCOL]])
                    w0, off = NCOL, 0
                else:
                    # tail: layer-1 finished, only column 32 is live
                    pp = m_ap(SW + 32, [[1, 1]])
                    rr = m_ap(SW + NCOL + 32, [[1, 1]])
                    w0, off = 1, 32
                if 1 <= tau <= 5:
                    # DVE-produced threshold column: rides the scan pipeline,
                    # no cross-engine semaphore
                    nc.vector.tensor_scalar(
                        bass.AP(Up.tensor, Up.offset + tau,
                                [[UW, PMAX], [TF, 32]]),
                        A1[:], float(c20[tau]), None, Alu.mult,
                    )
                # P = q*P + s_{tau-1}
                nc.vector.scalar_tensor_tensor(
                    pp, pp, q, m_ap(tau * NCOL + off, [[1, w0]]),
                    Alu.mult, Alu.add,
                )
                # R = q*R + P
                nc.vector.scalar_tensor_tensor(
                    rr, rr, q, pp, Alu.mult, Alu.add,
                )
                # s_{tau} = (R + 0.5) <= u/20
                nc.vector.scalar_tensor_tensor(
                    m_ap((tau + 1) * NCOL + off, [[1, w0]]),
                    rr, 0.5,
                    bass.AP(Up.tensor, Up.offset + off * TF + tau,
                            [[UW, PMAX], [TF, w0]]),
                    Alu.add, Alu.is_le,
                )

                if tau == OSPLIT - 1 + LAG and tau >= T:
                    nc.scalar.activation(
                        ostage[:, 0:OSPLIT],
                        m_ap((LAG + 1) * NCOL + 32, [[NCOL, OSPLIT]],
                             parts=84),
                        Act.Copy,
                    )
                    nc.sync.dma_start(out_d[:, 0:OSPLIT], ostage[:, 0:OSPLIT])
                if tau < T:
                    # PE fc2 block launch (waits on spike history via sems)
                    if tau in blk_at:
                        bi, tb0, tb1 = blk_at[tau]
                        blk = tb1 - tb0
                        a2ps = ps2.tile([NO2, B, NB], f32, tag="a2ps",
                                        name="a2ps")
                        a2ps_t[bi] = a2ps
                        for c in range(4):
                            nc.tensor.matmul(
                                a2ps[:, 0:blk, :],
                                w2sb[:, c, :],
                                m_ap((tb0 + 1) * NCOL + c * NB,
                                     [[NCOL, blk], [1, NB]]),
                                start=(c == 0), stop=(c == 3),
                            )
                    # threshold chunks stay 2 ahead of the scan
                    g0 = tau + 2 * UCHUNK
                    need_chunk = (tau == 1) or (tau % UCHUNK == 6 and g0 < T)
                    if need_chunk:
                        # gpsimd waits for op3(tau-1) before touching SBUF,
                        # keeping the chunk's port traffic off the hot steps
                        nc.gpsimd.tensor_copy(
                            gsc[0:1, 0:1],
                            bass.AP(M.tensor, MOFF + tau * NCOL,
                                    [[MW, 1], [1, 1]]),
                        )
                        if tau == 1:
                            emit_urange(18, 30)
                        else:
                            emit_urange(g0, min(g0 + UCHUNK, T))
                    if tau == OSPLIT - 1 + LAG:
                        # early output half overlaps the remaining tail
                        nc.scalar.activation(
                            ostage[:, 0:OSPLIT],
                            m_ap((LAG + 1) * NCOL + 32, [[NCOL, OSPLIT]],
                                 parts=84),
                            Act.Copy,
                        )
                        nc.sync.dma_start(out_d[:, 0:OSPLIT],
                                          ostage[:, 0:OSPLIT])
                    for bi, which in evac_at.get(tau, []):
                        emit_evac(bi, which)
                        if which == 1:
                            emit_restack(bi)
                    # DVE stream: delayed PSP scans
                    for bi in wscan_at.get(tau, []):
                        emit_wscan(bi)

            # ---- output: remaining layer-2 spikes ----
            nc.scalar.activation(
                ostage[:, OSPLIT:T],
                m_ap((OSPLIT + LAG + 1) * NCOL + 32, [[NCOL, T - OSPLIT]],
                     parts=84),
                Act.Copy,
            )
            nc.sync.dma_start(out_d[:, OSPLIT:T], ostage[:, OSPLIT:T])

    nc.compile()
    return nc


def _host_inputs(input, w1, w2):
    f32 = np.float32
    q, p, k2, c20 = _consts()
    flat = np.ascontiguousarray(input.reshape(64, -1).astype(f32))
    # fTp[p, k, n] = flat[n, k*128+p]
    fTp = np.ascontiguousarray(
        flat.T.reshape(KT, 128, 64).transpose(1, 0, 2))  # (128, KT, 64)
    # w1p[p, k*410+o] = w1[o, k*128+p]
    w1T = w1.astype(f32).T.reshape(KT, 128, NO1)          # (k, p, o)
    w1p = np.ascontiguousarray(
        w1T.transpose(1, 0, 2).reshape(128, KT * NO1))
    w2p = np.zeros((PMAX, 4, NO2), f32)
    for c in range(4):
        w2p[0:MC[c], c, :] = w2.astype(f32)[:, OFF[c]:OFF[c] + MC[c]].T
    pconst = np.full((84, B), p, f32)
    eye8 = np.eye(NB, dtype=f32)
    sel32 = np.zeros((NO2, 3, 32), f32)
    for j in range(3):
        for o2 in range(NO2):
            sel32[o2, j, 10 * j + o2] = 1.0
    c20rep = np.broadcast_to(c20, (PMAX, T)).copy()
    return fTp, w1p, w2p, pconst, eye8, sel32, c20rep


def kernel(input, w1, w2):
    from concourse.bass_utils import run_bass_kernel_spmd

    if "nc" not in _CACHE:
        _CACHE["nc"] = build()
    nc = _CACHE["nc"]

    fTp, w1p, w2p, pconst, eye8, sel32, c20rep = _host_inputs(input, w1, w2)
    in_maps = []
    for core in range(8):
        fTc = fTp[:, :, core * NB:(core + 1) * NB].reshape(128, KT * NB)
        in_maps.append({
            "fTp": np.ascontiguousarray(fTc),
            "w1p": w1p,
            "w2p": w2p,
            "pconst": pconst,
            "eye8": eye8,
            "sel32": sel32,
            "c20rep": c20rep,
        })
    res = run_bass_kernel_spmd(nc, in_maps, core_ids=list(range(8)))
    # row r = 32*g + 10*j + o2 holds batch n = 3*g + j
    rows = np.array([32 * (n // 3) + 10 * (n % 3) + np.arange(NO2)
                     for n in range(NB)])          # (8, 10)
    full = np.zeros((64, NO2, T), np.float32)
    for core in range(8):
        o = res.results[core]["out"]               # (84, T)
        full[core * NB:(core + 1) * NB] = o[rows]
    return full
